# revision 53
# baseline (speedup 1.0000x reference)
"""Trainium2 Bass kernel for nn_Attention (Quad2-normalized multi-head attention).

Problem: B=8, N=1024, C=768, H=12 heads, head_dim=64.
  qkv = x @ qkv_w.T ; per head: s = q @ k.T ; t = (s/8 + 5)^2
  attn = t / rowsum(t) ; out = attn @ v ; y = out @ proj_w.T + proj_b

Sharding: data-parallel over batch B — one batch element per NeuronCore (8 cores).

Per-core layout strategy (everything feature-major / transposed so the PE
contraction dim lands on partitions):
  xt = x[b].T [768, 1024] bf16; qkt = qkv_w[:1536] @ x.T in [feat, n] layout;
  V = x @ Wv.T in [n, feat] layout, ones-AUGMENTED (65 columns per head).
  Per head: scores st[m, n] = k @ q.T; t = (s+40)^2 evicted by ACT-Square /
  DVE (the 64x scale vs the reference's (s/8+5)^2 cancels in normalization);
  AV matmul with M=65 gives U.T[d, n] on psum rows 0:64 AND the row-sum
  r[n] on row 64 for free (ones column). The r rows are DMA-gathered to
  partitions {0,32} of a pair tile, reciprocal'd in one batched DVE op
  (reciprocal_approx_fast), broadcast across partitions with a K=1 ones
  matmul, and U/r is the DVE normalize multiply. The odd head's rows reach
  partitions 64:128 of the proj input tile via an SBUF->SBUF DMA.
  proj: y.T = Wp @ O.T -> DMA out [768, 1024] fp32; host transposes back.
Scheduling: scores of the two heads interleave (adjacent PE row strips);
the AV chain lags one m-tile behind the score evictions; each pair's QK^T
tiles are computed two pairs ahead inside the ACT-paced score windows;
normalization is deferred one pair; proj is split head/mid/last so its
matmuls fill the final normalize chain.
"""

import numpy as np

TRACE = False
TRACE_KWARGS = {}
LAST_RESULT = None

B, N, C = 8, 1024, 768
H, HD = 12, 64
NT = N // 128      # 8 n/m tiles
CT = C // 128      # 6 feature tiles
SC = 512           # psum-bank chunk of the free dim
# fraction of square-evictions routed to DVE instead of ACT: mt % DVE_MOD == DVE_REM
DVE_MOD = 99
DVE_REM = 5
T_BUFS = 20

_CACHE = {}


def _ensure_path():
    import sys
    for p in ("/opt/trn_rl_repo", "/root/.axon_site/_ro/trn_rl_repo"):
        if p not in sys.path:
            sys.path.insert(0, p)


def _build_nc(loop_n=None):
    import contextlib
    import concourse.bacc as bacc
    import concourse.mybir as mybir
    import concourse.tile as tile

    f32 = mybir.dt.float32
    bf16 = mybir.dt.bfloat16
    AF = mybir.ActivationFunctionType

    nc = bacc.Bacc("TRN2", target_bir_lowering=False)
    xt_d = nc.dram_tensor("xt", [C, N], bf16, kind="ExternalInput")
    wqk_d = nc.dram_tensor("wqk", [C, 2 * C], bf16, kind="ExternalInput")
    wv_d = nc.dram_tensor("wv", [C, C], bf16, kind="ExternalInput")
    wp_d = nc.dram_tensor("wp", [C, C], bf16, kind="ExternalInput")
    yt_d = nc.dram_tensor("yt", [C, N], f32, kind="ExternalOutput")

    with tile.TileContext(nc) as tc:
        with (
            tc.tile_pool(name="pw", bufs=1) as pw,
            tc.tile_pool(name="pq", bufs=1) as pq,
            tc.tile_pool(name="pt", bufs=T_BUFS) as pt,
            tc.tile_pool(name="pu", bufs=2) as pu,
            tc.tile_pool(name="psa", bufs=2, space="PSUM") as psa,
            tc.tile_pool(name="psu", bufs=2, space="PSUM") as psu,
        ):
            mm = nc.tensor.matmul

            bias40 = pw.tile([128, 1], f32, tag="bias40", name="bias40")
            ones_bc = pw.tile([128, 64], bf16, tag="ones_bc", name="ones_bc")
            nc.gpsimd.memset(bias40[:], 40.0)
            nc.gpsimd.memset(ones_bc[:], 1.0)

            # loop_n: benchmarking mode — run the whole body loop_n times on
            # device so execution time rises above the axon dispatch quantum
            loop_ctx = tc.For_i(0, loop_n, 1) if loop_n else contextlib.nullcontext()
            loop_stack = contextlib.ExitStack()
            loop_stack.enter_context(loop_ctx)

            # ---- input DMAs, interleaved in first-use order so the V-phase
            # k=0 matmul unblocks after ~2 transfers
            wv_sb = []
            xt_sb = []
            wqk_sb = []
            wp_sb = []
            for k in range(CT):
                t_ = pw.tile([128, N], bf16, tag=f"xt{k}", name=f"xt{k}")
                nc.sync.dma_start(out=t_[:], in_=xt_d[k * 128:(k + 1) * 128, :])
                xt_sb.append(t_)
                t_ = pw.tile([128, C], bf16, tag=f"wv{k}", name=f"wv{k}")
                nc.sync.dma_start(out=t_[:], in_=wv_d[k * 128:(k + 1) * 128, :])
                wv_sb.append(t_)
            for k in range(CT):
                t_ = pw.tile([128, 2 * C], bf16, tag=f"wqk{k}", name=f"wqk{k}")
                nc.sync.dma_start(out=t_[:], in_=wqk_d[k * 128:(k + 1) * 128, :])
                wqk_sb.append(t_)
            for k in range(CT):
                t_ = pw.tile([128, C], bf16, tag=f"wp{k}", name=f"wp{k}")
                nc.sync.dma_start(out=t_[:], in_=wp_d[k * 128:(k + 1) * 128, :])
                wp_sb.append(t_)

            # ---- V = x @ Wv.T, [n, f] layout, stored ones-augmented:
            # head h occupies columns 65h..65h+64 with a ones column at
            # 65h+64, so the AV matmul (M=65) also produces the row-sum r
            vv = []
            for mt in range(NT):
                ps = psa.tile([128, C], f32, tag="st", name=f"vps{mt}")
                for (o, w) in ((0, SC), (SC, C - SC)):
                    for k in range(CT):
                        mm(ps[:, o:o + w],
                           xt_sb[k][:, mt * 128:(mt + 1) * 128],
                           wv_sb[k][:, o:o + w],
                           start=(k == 0), stop=(k == CT - 1))
                v_ = pq.tile([128, H * 65], bf16, tag=f"vv{mt}", name=f"vv{mt}")
                # dense memset then strided copy: the skipped columns
                # (65h+64) stay 1.0 — the ones-augmentation
                nc.gpsimd.memset(v_[:], 1.0)
                vdst = v_[:, :].rearrange("p (h s) -> p h s", s=65)[:, :, 0:64]
                vsrc = ps[:, 0:C].rearrange("p (h s) -> p h s", s=64)
                nc.vector.tensor_copy(vdst, vsrc)
                vv.append(v_)

            # ---- QK^T = Wqk @ x.T, [feat, n] layout. Only pairs 0-1 are
            # computed upfront; later pairs' tiles are spliced into the
            # (ACT-paced) score loops two pairs ahead.
            qkt = [None] * 12

            def emit_qkt(mt):
                ps = psa.tile([128, N], f32, tag="st", name=f"qkps{mt}")
                for c in range(2):
                    for k in range(CT):
                        mm(ps[:, c * SC:(c + 1) * SC],
                           wqk_sb[k][:, mt * 128:(mt + 1) * 128],
                           xt_sb[k][:, c * SC:(c + 1) * SC],
                           start=(k == 0), stop=(k == CT - 1))
                q_ = pq.tile([128, N], bf16, tag=f"qkt{mt % 4}", bufs=2,
                             name=f"qkt{mt}")
                nc.vector.tensor_copy(q_[:], ps[:])
                qkt[mt] = q_

            for p0 in (0, 1):
                emit_qkt(p0)
                emit_qkt(6 + p0)

            # ---- attention, one head at a time. Per head the AV matmul has
            # M=65 (64 v-dims + ones column): psum row 64 = row-sum r. The r
            # rows are DMA-gathered (partition move) into rows {0,32} of a
            # per-pair tile for a batched reciprocal; normalization is
            # deferred one pair so the reciprocal pipeline hides under the
            # next pair's scores.
            ot = [None] * 6
            deferred = None

            def emit_normalize(d):
                # broadcast 1/r across partitions with a K=1 ones matmul
                # (bf16, ~213ns per 512 chunk), then normalize on the DVE
                p_, usbs_, rr_ = d
                o_ = pq.tile([128, N], bf16, tag=f"ot{p_}", name=f"ot{p_}")
                ottmp = pu.tile([64, N], bf16, tag="ottmp", bufs=2,
                                name=f"ottmp{p_}")
                for hi in (0, 1):
                    row = hi * 32
                    bc = psu.tile([64, N], f32, tag="u", name=f"bc{p_}_{hi}")
                    for c in range(2):
                        cs = slice(c * SC, (c + 1) * SC)
                        mm(bc[:, cs], ones_bc[row:row + 1, :],
                           rr_[row:row + 1, cs],
                           start=True, stop=True, tile_position=(row, 0))
                    dst = o_[0:64, :] if hi == 0 else ottmp[:, :]
                    nc.vector.tensor_mul(dst, usbs_[hi][0:64, :], bc[:, :])
                # odd head's rows move to partitions 64:128 via DMA
                nc.sync.dma_start(out=o_[64:128, :], in_=ottmp[:, :])
                ot[p_] = o_

            for p in range(6):
                qa = qkt[p]
                ka = qkt[6 + p]
                rg = pu.tile([33, N], f32, tag="rg", bufs=2, name=f"rg{p}")
                usbs = {}
                ts = {}
                ups = {}

                def emit_av(hi, k, c):
                    cs = slice(c * SC, (c + 1) * SC)
                    h = 2 * p + hi
                    mm(ups[hi][:, cs],
                       vv[k][:, h * 65:h * 65 + 65],
                       ts[(k, hi * 64)][:, cs],
                       start=(k == 0), stop=(k == NT - 1))

                def finish_head(hi):
                    # DVE (not ACT) so the eviction never queues ahead of the
                    # next score-square on the ACT engine
                    usb = pu.tile([65, N], f32, tag="usb", bufs=4,
                                  name=f"usb{p}_{hi}")
                    nc.vector.tensor_copy(usb[:], ups[hi][:])
                    nc.sync.dma_start(out=rg[hi * 32:hi * 32 + 1, :],
                                      in_=usb[64:65, :])
                    usbs[hi] = usb

                # scores for both heads interleaved per m-tile (adjacent row
                # strips overlap on the PE); head A's AV chunk-0 chain is
                # lag-interleaved, the rest of the AV work runs densely after
                for mt in range(NT):
                    for hoff in (0, 64):
                        ps = psa.tile([128, N], f32, tag="st",
                                      name=f"sps{p}_{mt}_{hoff}")
                        for c in range(2):
                            cs = slice(c * SC, (c + 1) * SC)
                            mm(ps[:, cs],
                               ka[hoff:hoff + 64, mt * 128:(mt + 1) * 128],
                               qa[hoff:hoff + 64, cs],
                               start=True, stop=True)
                        t_ = pt.tile([128, N], bf16, tag="t",
                                     name=f"t{p}_{mt}_{hoff}")
                        if mt % DVE_MOD == DVE_REM:
                            # bf16 intermediate: the mult then runs in the
                            # DVE 2x packed mode (half the cycles)
                            tmp = pu.tile([128, N], bf16, tag="sqtmp", bufs=2,
                                          name=f"tmp{p}_{mt}_{hoff}")
                            nc.vector.tensor_scalar_add(tmp[:], ps[:], 40.0)
                            nc.vector.tensor_mul(t_[:], tmp[:], tmp[:])
                        else:
                            nc.scalar.activation(t_[:], ps[:], AF.Square,
                                                 bias=bias40[:], scale=1.0)
                        ts[(mt, hoff)] = t_
                    if mt == 0:
                        if deferred is not None:
                            emit_normalize(deferred)
                            deferred = None
                        ups[0] = psu.tile([65, N], f32, tag="u", bufs=2,
                                          name=f"uA{p}")
                    elif mt == 1:
                        pass  # lag so head A's AV doesn't stall on slot reuse
                    elif mt == 2:
                        emit_av(0, 0, 0)
                        emit_av(0, 1, 0)
                    else:
                        emit_av(0, mt - 1, 0)
                    # stream pair p+2's QK^T tiles into the ACT-paced window
                    if mt == 3 and p + 2 < 6:
                        emit_qkt(p + 2)
                    elif mt == 6 and p + 2 < 6:
                        emit_qkt(6 + p + 2)
                emit_av(0, NT - 1, 0)
                for k in range(NT):
                    emit_av(0, k, 1)
                finish_head(0)
                ups[1] = psu.tile([65, N], f32, tag="u", bufs=2,
                                  name=f"uB{p}")
                for k in range(NT):
                    emit_av(1, k, 0)
                    emit_av(1, k, 1)
                finish_head(1)
                rrf = pu.tile([33, N], f32, tag="rrf", bufs=2, name=f"rrf{p}")
                nc.vector.reciprocal_approx_fast(rrf[:], rg[:])
                rr = pu.tile([33, N], bf16, tag="rr", bufs=2, name=f"rr{p}")
                nc.vector.tensor_copy(rr[:], rrf[:])
                deferred = (p, usbs, rr)

            # ---- proj: y.T = Wp @ O.T  (uses the "st" slots: double-buffered).
            # Two-phase emission: f=0..3 matmuls (whose ot tiles are ready
            # early) for tile ct are issued before f=4,5 of tile ct-1, so the
            # PE has work while the last pair's normalization completes.
            yps = [None] * CT

            def proj_head(ct, pool, tag):
                yps[ct] = pool.tile([128, N], f32, tag=tag, name=f"yps{ct}")
                for c in range(2):
                    cs = slice(c * SC, (c + 1) * SC)
                    for f in range(CT - 2):
                        mm(yps[ct][:, cs],
                           wp_sb[f][:, ct * 128:(ct + 1) * 128],
                           ot[f][:, cs],
                           start=(f == 0), stop=False)

            def proj_mid(ct):
                # f=4 contraction tile: ot[4] is ready one pair early
                f = CT - 2
                for c in range(2):
                    cs = slice(c * SC, (c + 1) * SC)
                    mm(yps[ct][:, cs],
                       wp_sb[f][:, ct * 128:(ct + 1) * 128],
                       ot[f][:, cs],
                       start=False, stop=False)

            def proj_last(ct):
                f = CT - 1
                ysb = pu.tile([128, N], f32, tag="ysb", bufs=2, name=f"ysb{ct}")
                for c in range(2):
                    cs = slice(c * SC, (c + 1) * SC)
                    mm(yps[ct][:, cs],
                       wp_sb[f][:, ct * 128:(ct + 1) * 128],
                       ot[f][:, cs],
                       start=False, stop=True)
                    # chunked eviction + store so the last DMA isn't serialized
                    # behind a full-tile eviction
                    nc.scalar.copy(ysb[:, cs], yps[ct][:, cs])
                    eng = nc.sync if ct % 2 == 0 else nc.gpsimd
                    eng.dma_start(out=yt_d[ct * 128:(ct + 1) * 128, cs],
                                  in_=ysb[:, cs])

            # proj heads (f=0..3 only read pairs 0-3) are emitted before the
            # final pair's normalization: 4 of them fit in the st+u slots and
            # the f=4 column also only needs ot[4], so the PE streams ~9us of
            # matmuls while the last recip/broadcast chain completes
            proj_head(0, psa, "st")
            proj_head(1, psa, "st")
            # the final normalize's broadcast tiles must take "u" slots
            # before proj heads 2/3 do (else slot-cycle deadlock)
            emit_normalize(deferred)
            proj_head(2, psu, "u")
            proj_head(3, psu, "u")
            for ct in range(4):
                proj_mid(ct)
            proj_last(0)
            proj_last(1)
            proj_head(4, psa, "st")
            proj_mid(4)
            proj_last(2)
            proj_head(5, psa, "st")
            proj_mid(5)
            proj_last(3)
            proj_last(4)
            proj_last(5)

            loop_stack.close()

    nc.compile()
    return nc


def _make_runner(nc):
    """Build the 8-core sharded jitted executable once (cached across calls)."""
    import jax
    import concourse.mybir as mybir
    from concourse import bass2jax
    from jax.experimental.shard_map import shard_map
    from jax.sharding import Mesh, PartitionSpec

    bass2jax.install_neuronx_cc_hook()
    partition_name = nc.partition_id_tensor.name if nc.partition_id_tensor else None
    in_names, out_names, out_avals, zero_outs = [], [], [], []
    for alloc in nc.m.functions[0].allocations:
        if not isinstance(alloc, mybir.MemoryLocationSet):
            continue
        name = alloc.memorylocations[0].name
        if alloc.kind == "ExternalInput":
            if name != partition_name:
                in_names.append(name)
        elif alloc.kind == "ExternalOutput":
            shape = tuple(alloc.tensor_shape)
            dtype = mybir.dt.np(alloc.dtype)
            out_names.append(name)
            out_avals.append(jax.core.ShapedArray(shape, dtype))
            zero_outs.append(np.zeros((B * shape[0], *shape[1:]), dtype))
    all_in_names = list(in_names) + list(out_names)
    if partition_name is not None:
        all_in_names.append(partition_name)

    def _body(*args):
        operands = list(args)
        if partition_name is not None:
            operands.append(bass2jax.partition_id_tensor())
        outs = bass2jax._bass_exec_p.bind(
            *operands,
            out_avals=tuple(out_avals),
            in_names=tuple(all_in_names),
            out_names=tuple(out_names),
            lowering_input_output_aliases=(),
            sim_require_finite=True,
            sim_require_nnan=True,
            nc=nc,
        )
        return tuple(outs)

    devices = jax.devices()[:B]
    mesh = Mesh(np.asarray(devices), ("core",))
    n_io = len(in_names) + len(out_avals)
    fn = jax.jit(shard_map(_body, mesh=mesh,
                           in_specs=(PartitionSpec("core"),) * n_io,
                           out_specs=(PartitionSpec("core"),) * len(out_avals),
                           check_rep=False))
    return fn, in_names, out_names, zero_outs


def kernel(x, qkv_w, proj_w, proj_b):
    global LAST_RESULT
    _ensure_path()
    import ml_dtypes

    bf16 = ml_dtypes.bfloat16
    x = np.asarray(x, dtype=np.float32)
    qkv_w = np.asarray(qkv_w, dtype=np.float32)
    proj_w = np.asarray(proj_w, dtype=np.float32)
    proj_b = np.asarray(proj_b, dtype=np.float32)

    if "runner" not in _CACHE:
        _CACHE["nc"] = _build_nc()
        _CACHE["runner"] = _make_runner(_CACHE["nc"])
    fn, in_names, out_names, zero_outs = _CACHE["runner"]

    wqk = np.ascontiguousarray(qkv_w[:2 * C].T).astype(bf16)
    wv = np.ascontiguousarray(qkv_w[2 * C:].T).astype(bf16)
    wp = np.ascontiguousarray(proj_w.T).astype(bf16)
    per_core = {
        "xt": np.concatenate(
            [np.ascontiguousarray(x[b].T).astype(bf16) for b in range(B)], axis=0),
        "wqk": np.concatenate([wqk] * B, axis=0),
        "wv": np.concatenate([wv] * B, axis=0),
        "wp": np.concatenate([wp] * B, axis=0),
    }
    args = [per_core[nm] for nm in in_names] + list(zero_outs)
    outs = fn(*args)
    yt = np.asarray(outs[out_names.index("yt")]).reshape(B, C, N)

    y = np.empty((B, N, C), dtype=np.float32)
    for b in range(B):
        y[b] = yt[b].T
    y += proj_b[None, None, :]
    return y


# revision 54
# speedup vs baseline: 1.0999x; 1.0999x over previous
"""Trainium2 Bass kernel for nn_Attention (Quad2-normalized multi-head attention).

Problem: B=8, N=1024, C=768, H=12 heads, head_dim=64.
  qkv = x @ qkv_w.T ; per head: s = q @ k.T ; t = (s/8 + 5)^2
  attn = t / rowsum(t) ; out = attn @ v ; y = out @ proj_w.T + proj_b

Sharding: data-parallel over batch B — one batch element per NeuronCore (8 cores).

Per-core layout strategy (everything feature-major / transposed so the PE
contraction dim lands on partitions):
  xt = x[b].T [768, 1024] bf16; qkt = qkv_w[:1536] @ x.T in [feat, n] layout;
  V = x @ Wv.T in [n, feat] layout, ones-AUGMENTED (65 columns per head).
  Per head: scores st[m, n] = k @ q.T; t = (s+40)^2 evicted by ACT-Square /
  DVE (the 64x scale vs the reference's (s/8+5)^2 cancels in normalization);
  AV matmul with M=65 gives U.T[d, n] on psum rows 0:64 AND the row-sum
  r[n] on row 64 for free (ones column). The r rows are DMA-gathered to
  partitions {0,32} of a pair tile, reciprocal'd in one batched DVE op
  (reciprocal_approx_fast), broadcast across partitions with a K=1 ones
  matmul, and U/r is the DVE normalize multiply. The odd head's rows reach
  partitions 64:128 of the proj input tile via an SBUF->SBUF DMA.
  proj: y.T = Wp @ O.T -> DMA out [768, 1024] fp32; host transposes back.
Scheduling: scores of the two heads interleave (adjacent PE row strips);
the AV chain lags one m-tile behind the score evictions; each pair's QK^T
tiles are computed two pairs ahead inside the ACT-paced score windows;
normalization is deferred one pair; proj is split head/mid/last so its
matmuls fill the final normalize chain.
"""

import numpy as np

TRACE = False
TRACE_KWARGS = {}
LAST_RESULT = None

B, N, C = 8, 1024, 768
H, HD = 12, 64
NT = N // 128      # 8 n/m tiles
CT = C // 128      # 6 feature tiles
SC = 512           # psum-bank chunk of the free dim
# fraction of square-evictions routed to DVE instead of ACT: mt % DVE_MOD == DVE_REM
DVE_MOD = 8
DVE_REM = 5
T_BUFS = 20

_CACHE = {}


def _ensure_path():
    import sys
    for p in ("/opt/trn_rl_repo", "/root/.axon_site/_ro/trn_rl_repo"):
        if p not in sys.path:
            sys.path.insert(0, p)


def _build_nc(loop_n=None):
    import contextlib
    import concourse.bacc as bacc
    import concourse.mybir as mybir
    import concourse.tile as tile

    f32 = mybir.dt.float32
    bf16 = mybir.dt.bfloat16
    AF = mybir.ActivationFunctionType

    nc = bacc.Bacc("TRN2", target_bir_lowering=False)
    xt_d = nc.dram_tensor("xt", [C, N], bf16, kind="ExternalInput")
    wqk_d = nc.dram_tensor("wqk", [C, 2 * C], bf16, kind="ExternalInput")
    wv_d = nc.dram_tensor("wv", [C, C], bf16, kind="ExternalInput")
    wp_d = nc.dram_tensor("wp", [C, C], bf16, kind="ExternalInput")
    yt_d = nc.dram_tensor("yt", [C, N], f32, kind="ExternalOutput")

    with tile.TileContext(nc) as tc:
        with (
            tc.tile_pool(name="pw", bufs=1) as pw,
            tc.tile_pool(name="pq", bufs=1) as pq,
            tc.tile_pool(name="pt", bufs=T_BUFS) as pt,
            tc.tile_pool(name="pu", bufs=2) as pu,
            tc.tile_pool(name="psa", bufs=2, space="PSUM") as psa,
            tc.tile_pool(name="psu", bufs=2, space="PSUM") as psu,
        ):
            mm = nc.tensor.matmul

            bias40 = pw.tile([128, 1], f32, tag="bias40", name="bias40")
            ones_bc = pw.tile([128, 64], bf16, tag="ones_bc", name="ones_bc")
            nc.gpsimd.memset(bias40[:], 40.0)
            nc.gpsimd.memset(ones_bc[:], 1.0)

            # loop_n: benchmarking mode — run the whole body loop_n times on
            # device so execution time rises above the axon dispatch quantum
            loop_ctx = tc.For_i(0, loop_n, 1) if loop_n else contextlib.nullcontext()
            loop_stack = contextlib.ExitStack()
            loop_stack.enter_context(loop_ctx)

            # ---- input DMAs, interleaved in first-use order so the V-phase
            # k=0 matmul unblocks after ~2 transfers
            wv_sb = []
            xt_sb = []
            wqk_sb = []
            wp_sb = []
            for k in range(CT):
                t_ = pw.tile([128, N], bf16, tag=f"xt{k}", name=f"xt{k}")
                nc.sync.dma_start(out=t_[:], in_=xt_d[k * 128:(k + 1) * 128, :])
                xt_sb.append(t_)
                t_ = pw.tile([128, C], bf16, tag=f"wv{k}", name=f"wv{k}")
                nc.sync.dma_start(out=t_[:], in_=wv_d[k * 128:(k + 1) * 128, :])
                wv_sb.append(t_)
            for k in range(CT):
                t_ = pw.tile([128, 2 * C], bf16, tag=f"wqk{k}", name=f"wqk{k}")
                nc.sync.dma_start(out=t_[:], in_=wqk_d[k * 128:(k + 1) * 128, :])
                wqk_sb.append(t_)
            for k in range(CT):
                t_ = pw.tile([128, C], bf16, tag=f"wp{k}", name=f"wp{k}")
                nc.sync.dma_start(out=t_[:], in_=wp_d[k * 128:(k + 1) * 128, :])
                wp_sb.append(t_)

            # ---- V = x @ Wv.T, [n, f] layout, stored ones-augmented:
            # head h occupies columns 65h..65h+64 with a ones column at
            # 65h+64, so the AV matmul (M=65) also produces the row-sum r
            vv = []
            for mt in range(NT):
                ps = psa.tile([128, C], f32, tag="st", name=f"vps{mt}")
                for (o, w) in ((0, SC), (SC, C - SC)):
                    for k in range(CT):
                        mm(ps[:, o:o + w],
                           xt_sb[k][:, mt * 128:(mt + 1) * 128],
                           wv_sb[k][:, o:o + w],
                           start=(k == 0), stop=(k == CT - 1))
                v_ = pq.tile([128, H * 65], bf16, tag=f"vv{mt}", name=f"vv{mt}")
                # dense memset then strided copy: the skipped columns
                # (65h+64) stay 1.0 — the ones-augmentation
                nc.gpsimd.memset(v_[:], 1.0)
                vdst = v_[:, :].rearrange("p (h s) -> p h s", s=65)[:, :, 0:64]
                vsrc = ps[:, 0:C].rearrange("p (h s) -> p h s", s=64)
                nc.vector.tensor_copy(vdst, vsrc)
                vv.append(v_)

            # ---- QK^T = Wqk @ x.T, [feat, n] layout. Only pairs 0-1 are
            # computed upfront; later pairs' tiles are spliced into the
            # (ACT-paced) score loops two pairs ahead.
            qkt = [None] * 12

            def emit_qkt(mt):
                ps = psa.tile([128, N], f32, tag="st", name=f"qkps{mt}")
                for c in range(2):
                    for k in range(CT):
                        mm(ps[:, c * SC:(c + 1) * SC],
                           wqk_sb[k][:, mt * 128:(mt + 1) * 128],
                           xt_sb[k][:, c * SC:(c + 1) * SC],
                           start=(k == 0), stop=(k == CT - 1))
                q_ = pq.tile([128, N], bf16, tag=f"qkt{mt % 4}", bufs=2,
                             name=f"qkt{mt}")
                nc.vector.tensor_copy(q_[:], ps[:])
                qkt[mt] = q_

            for p0 in (0, 1):
                emit_qkt(p0)
                emit_qkt(6 + p0)

            # ---- attention, one head at a time. Per head the AV matmul has
            # M=65 (64 v-dims + ones column): psum row 64 = row-sum r. The r
            # rows are DMA-gathered (partition move) into rows {0,32} of a
            # per-pair tile for a batched reciprocal; normalization is
            # deferred one pair so the reciprocal pipeline hides under the
            # next pair's scores.
            ot = [None] * 6
            deferred = None

            def emit_normalize(d):
                # broadcast 1/r across partitions with a K=1 ones matmul
                # (bf16, ~213ns per 512 chunk), then normalize on the DVE
                p_, usbs_, rr_ = d
                o_ = pq.tile([128, N], bf16, tag=f"ot{p_}", name=f"ot{p_}")
                ottmp = pu.tile([64, N], bf16, tag="ottmp", bufs=2,
                                name=f"ottmp{p_}")
                for hi in (0, 1):
                    row = hi * 32
                    bc = psu.tile([64, N], f32, tag="u", name=f"bc{p_}_{hi}")
                    for c in range(2):
                        cs = slice(c * SC, (c + 1) * SC)
                        mm(bc[:, cs], ones_bc[row:row + 1, :],
                           rr_[row:row + 1, cs],
                           start=True, stop=True, tile_position=(row, 0))
                    dst = o_[0:64, :] if hi == 0 else ottmp[:, :]
                    nc.vector.tensor_mul(dst, usbs_[hi][0:64, :], bc[:, :])
                # odd head's rows move to partitions 64:128 via DMA
                nc.sync.dma_start(out=o_[64:128, :], in_=ottmp[:, :])
                ot[p_] = o_

            for p in range(6):
                qa = qkt[p]
                ka = qkt[6 + p]
                rg = pu.tile([33, N], f32, tag="rg", bufs=2, name=f"rg{p}")
                usbs = {}
                ts = {}
                ups = {}

                def emit_av(hi, k, c):
                    cs = slice(c * SC, (c + 1) * SC)
                    h = 2 * p + hi
                    mm(ups[hi][:, cs],
                       vv[k][:, h * 65:h * 65 + 65],
                       ts[(k, hi * 64)][:, cs],
                       start=(k == 0), stop=(k == NT - 1))

                def finish_head(hi):
                    # DVE (not ACT) so the eviction never queues ahead of the
                    # next score-square on the ACT engine
                    usb = pu.tile([65, N], f32, tag="usb", bufs=4,
                                  name=f"usb{p}_{hi}")
                    nc.vector.tensor_copy(usb[:], ups[hi][:])
                    nc.sync.dma_start(out=rg[hi * 32:hi * 32 + 1, :],
                                      in_=usb[64:65, :])
                    usbs[hi] = usb

                # scores for both heads interleaved per m-tile (adjacent row
                # strips overlap on the PE); head A's AV chunk-0 chain is
                # lag-interleaved, the rest of the AV work runs densely after
                for mt in range(NT):
                    for hoff in (0, 64):
                        ps = psa.tile([128, N], f32, tag="st",
                                      name=f"sps{p}_{mt}_{hoff}")
                        for c in range(2):
                            cs = slice(c * SC, (c + 1) * SC)
                            mm(ps[:, cs],
                               ka[hoff:hoff + 64, mt * 128:(mt + 1) * 128],
                               qa[hoff:hoff + 64, cs],
                               start=True, stop=True)
                        t_ = pt.tile([128, N], bf16, tag="t",
                                     name=f"t{p}_{mt}_{hoff}")
                        if mt % DVE_MOD == DVE_REM:
                            # bf16 intermediate: the mult then runs in the
                            # DVE 2x packed mode (half the cycles)
                            tmp = pu.tile([128, N], bf16, tag="sqtmp", bufs=2,
                                          name=f"tmp{p}_{mt}_{hoff}")
                            nc.vector.tensor_scalar_add(tmp[:], ps[:], 40.0)
                            nc.vector.tensor_mul(t_[:], tmp[:], tmp[:])
                        else:
                            nc.scalar.activation(t_[:], ps[:], AF.Square,
                                                 bias=bias40[:], scale=1.0)
                        ts[(mt, hoff)] = t_
                    if mt == 0:
                        if deferred is not None:
                            emit_normalize(deferred)
                            deferred = None
                        ups[0] = psu.tile([65, N], f32, tag="u", bufs=2,
                                          name=f"uA{p}")
                    elif mt == 1:
                        pass  # lag so head A's AV doesn't stall on slot reuse
                    elif mt == 2:
                        emit_av(0, 0, 0)
                        emit_av(0, 1, 0)
                    else:
                        emit_av(0, mt - 1, 0)
                    # stream pair p+2's QK^T tiles into the ACT-paced window
                    if mt == 3 and p + 2 < 6:
                        emit_qkt(p + 2)
                    elif mt == 6 and p + 2 < 6:
                        emit_qkt(6 + p + 2)
                emit_av(0, NT - 1, 0)
                for k in range(NT):
                    emit_av(0, k, 1)
                finish_head(0)
                ups[1] = psu.tile([65, N], f32, tag="u", bufs=2,
                                  name=f"uB{p}")
                for k in range(NT):
                    emit_av(1, k, 0)
                    emit_av(1, k, 1)
                finish_head(1)
                rrf = pu.tile([33, N], f32, tag="rrf", bufs=2, name=f"rrf{p}")
                nc.vector.reciprocal_approx_fast(rrf[:], rg[:])
                rr = pu.tile([33, N], bf16, tag="rr", bufs=2, name=f"rr{p}")
                nc.vector.tensor_copy(rr[:], rrf[:])
                deferred = (p, usbs, rr)

            # ---- proj: y.T = Wp @ O.T  (uses the "st" slots: double-buffered).
            # Two-phase emission: f=0..3 matmuls (whose ot tiles are ready
            # early) for tile ct are issued before f=4,5 of tile ct-1, so the
            # PE has work while the last pair's normalization completes.
            yps = [None] * CT

            def proj_head(ct, pool, tag):
                yps[ct] = pool.tile([128, N], f32, tag=tag, name=f"yps{ct}")
                for c in range(2):
                    cs = slice(c * SC, (c + 1) * SC)
                    for f in range(CT - 2):
                        mm(yps[ct][:, cs],
                           wp_sb[f][:, ct * 128:(ct + 1) * 128],
                           ot[f][:, cs],
                           start=(f == 0), stop=False)

            def proj_mid(ct):
                # f=4 contraction tile: ot[4] is ready one pair early
                f = CT - 2
                for c in range(2):
                    cs = slice(c * SC, (c + 1) * SC)
                    mm(yps[ct][:, cs],
                       wp_sb[f][:, ct * 128:(ct + 1) * 128],
                       ot[f][:, cs],
                       start=False, stop=False)

            def proj_last(ct):
                f = CT - 1
                ysb = pu.tile([128, N], f32, tag="ysb", bufs=2, name=f"ysb{ct}")
                for c in range(2):
                    cs = slice(c * SC, (c + 1) * SC)
                    mm(yps[ct][:, cs],
                       wp_sb[f][:, ct * 128:(ct + 1) * 128],
                       ot[f][:, cs],
                       start=False, stop=True)
                    # chunked eviction + store so the last DMA isn't serialized
                    # behind a full-tile eviction
                    nc.scalar.copy(ysb[:, cs], yps[ct][:, cs])
                    eng = nc.sync if ct % 2 == 0 else nc.gpsimd
                    eng.dma_start(out=yt_d[ct * 128:(ct + 1) * 128, cs],
                                  in_=ysb[:, cs])

            # proj heads (f=0..3 only read pairs 0-3) are emitted before the
            # final pair's normalization: 4 of them fit in the st+u slots and
            # the f=4 column also only needs ot[4], so the PE streams ~9us of
            # matmuls while the last recip/broadcast chain completes
            proj_head(0, psa, "st")
            proj_head(1, psa, "st")
            # the final normalize's broadcast tiles must take "u" slots
            # before proj heads 2/3 do (else slot-cycle deadlock)
            emit_normalize(deferred)
            proj_head(2, psu, "u")
            proj_head(3, psu, "u")
            for ct in range(4):
                proj_mid(ct)
            proj_last(0)
            proj_last(1)
            proj_head(4, psa, "st")
            proj_mid(4)
            proj_last(2)
            proj_head(5, psa, "st")
            proj_mid(5)
            proj_last(3)
            proj_last(4)
            proj_last(5)

            loop_stack.close()

    nc.compile()
    return nc


def _make_runner(nc):
    """Build the 8-core sharded jitted executable once (cached across calls)."""
    import jax
    import concourse.mybir as mybir
    from concourse import bass2jax
    from jax.experimental.shard_map import shard_map
    from jax.sharding import Mesh, PartitionSpec

    bass2jax.install_neuronx_cc_hook()
    partition_name = nc.partition_id_tensor.name if nc.partition_id_tensor else None
    in_names, out_names, out_avals, zero_outs = [], [], [], []
    for alloc in nc.m.functions[0].allocations:
        if not isinstance(alloc, mybir.MemoryLocationSet):
            continue
        name = alloc.memorylocations[0].name
        if alloc.kind == "ExternalInput":
            if name != partition_name:
                in_names.append(name)
        elif alloc.kind == "ExternalOutput":
            shape = tuple(alloc.tensor_shape)
            dtype = mybir.dt.np(alloc.dtype)
            out_names.append(name)
            out_avals.append(jax.core.ShapedArray(shape, dtype))
            zero_outs.append(np.zeros((B * shape[0], *shape[1:]), dtype))
    all_in_names = list(in_names) + list(out_names)
    if partition_name is not None:
        all_in_names.append(partition_name)

    def _body(*args):
        operands = list(args)
        if partition_name is not None:
            operands.append(bass2jax.partition_id_tensor())
        outs = bass2jax._bass_exec_p.bind(
            *operands,
            out_avals=tuple(out_avals),
            in_names=tuple(all_in_names),
            out_names=tuple(out_names),
            lowering_input_output_aliases=(),
            sim_require_finite=True,
            sim_require_nnan=True,
            nc=nc,
        )
        return tuple(outs)

    devices = jax.devices()[:B]
    mesh = Mesh(np.asarray(devices), ("core",))
    n_io = len(in_names) + len(out_avals)
    fn = jax.jit(shard_map(_body, mesh=mesh,
                           in_specs=(PartitionSpec("core"),) * n_io,
                           out_specs=(PartitionSpec("core"),) * len(out_avals),
                           check_rep=False))
    return fn, in_names, out_names, zero_outs


def kernel(x, qkv_w, proj_w, proj_b):
    global LAST_RESULT
    _ensure_path()
    import ml_dtypes

    bf16 = ml_dtypes.bfloat16
    x = np.asarray(x, dtype=np.float32)
    qkv_w = np.asarray(qkv_w, dtype=np.float32)
    proj_w = np.asarray(proj_w, dtype=np.float32)
    proj_b = np.asarray(proj_b, dtype=np.float32)

    if "runner" not in _CACHE:
        _CACHE["nc"] = _build_nc()
        _CACHE["runner"] = _make_runner(_CACHE["nc"])
    fn, in_names, out_names, zero_outs = _CACHE["runner"]

    wqk = np.ascontiguousarray(qkv_w[:2 * C].T).astype(bf16)
    wv = np.ascontiguousarray(qkv_w[2 * C:].T).astype(bf16)
    wp = np.ascontiguousarray(proj_w.T).astype(bf16)
    per_core = {
        "xt": np.concatenate(
            [np.ascontiguousarray(x[b].T).astype(bf16) for b in range(B)], axis=0),
        "wqk": np.concatenate([wqk] * B, axis=0),
        "wv": np.concatenate([wv] * B, axis=0),
        "wp": np.concatenate([wp] * B, axis=0),
    }
    args = [per_core[nm] for nm in in_names] + list(zero_outs)
    outs = fn(*args)
    yt = np.asarray(outs[out_names.index("yt")]).reshape(B, C, N)

    y = np.empty((B, N, C), dtype=np.float32)
    for b in range(B):
        y[b] = yt[b].T
    y += proj_b[None, None, :]
    return y
